# revision 13
# baseline (speedup 1.0000x reference)
"""BiLSTM + additive-attention pooling kernel for Trainium2 (8 NeuronCores).

Strategy (data-parallel over batch, per spec sharding hint):
  - B=64 batch rows sharded 8-ways -> BL=8 rows per core; LSTM/attention
    weights replicated.  Each core runs BOTH directions of the LSTM as two
    independent instruction chains (latency hiding) over S=2048 steps.
  - Transposed layout everywhere: hidden dim on SBUF partitions, batch on the
    free dim.  Recurrent matmuls: z_G^T = U_G^T @ h^T with U gate-chunks as
    the stationary operand (fp16 -> fast weight load), h^T (fp16) streaming.
  - The input projection x@W and the bias are folded into the same PSUM banks
    the recurrent matmuls accumulate into:  per 16-step bank we first do a
    rank-1 bias matmul (start=True) + W-chunk matmuls over an x slab, then the
    per-step U-chunk matmuls land on top with start=False.  No xW round-trip
    through DRAM, no extra vector-engine adds.
  - Gates use the sigmoid-only trick: tanh(z) = 2*sigmoid(2z)-1, with the
    factor 2 folded into the g-gate columns of U/W/b on the host.  One
    sigmoid ACT per (dir, step) covers all 4 gates; the cell state is tracked
    halved (C = c/2) so the correction is a single fused scalar_tensor_tensor:
        m   = (sig_g - 0.5) * i          # = i*g/2
        u   = f * C_prev                 # C' = f*C_prev + m
        th  = tanh(2*C')                 # = tanh(c)  (ACT, scale=2)
        h   = o * th                     # fp16, feeds next matmul + attention
  - h^T for all t is spilled to DRAM (fp16) in 64-step rings, then the
    attention phase (big matmuls, softmax, weighted sum) runs on-chip.

kernel(**inputs) takes the FULL fp32 inputs and returns the FULL [64, 256]
fp32 output; it shards/preps on the host, runs the same program SPMD on cores
0-7 via run_bass_kernel_spmd, and concatenates the per-core outputs.
"""

import sys

for p in ("/opt/trn_rl_repo", "/opt/pypackages"):
    if p not in sys.path:
        sys.path.insert(0, p)

from contextlib import ExitStack

import numpy as np

import concourse.bass as bass
import concourse.tile as tile
from concourse import bacc, mybir
from concourse.bass import ds

F32 = mybir.dt.float32
F16 = mybir.dt.float16
AF = mybir.ActivationFunctionType
ALU = mybir.AluOpType

B, S, D, H, O = 64, 2048, 128, 128, 256
NCORES = 8
BL = B // NCORES  # batch rows per core

_PROGRAM_CACHE = {}


def _ap(t, offset, ap):
    """Manual access-pattern helper."""
    return bass.AP(tensor=t.tensor, offset=t.offset + offset, ap=ap)


def build_program(s_len=S, bl=BL, t_bank=16, n_bank_pairs=2, unroll=False,
                  dyn_dma_engine="sync"):
    """Build the per-core Bass program.

    s_len steps, bl batch rows.  Each PSUM bank holds t_bank steps of one
    direction; one For_i iteration covers n_bank_pairs banks per direction
    (t_body = t_bank * n_bank_pairs steps).  unroll=True replaces the
    dynamic For_i loop with a fully unrolled python loop (static offsets).
    """
    t_body = t_bank * n_bank_pairs
    assert s_len % t_body == 0
    iters = s_len // t_body
    CW = t_bank * bl  # psum cols per gate chunk (16*8 = 128)

    nc = bacc.Bacc("TRN2", target_bir_lowering=False, debug=False)

    x2 = nc.dram_tensor("x2", [D, s_len, bl], F16, kind="ExternalInput").ap()
    Ucat = nc.dram_tensor("Ucat", [H, 8 * H], F16, kind="ExternalInput").ap()
    Wcat = nc.dram_tensor("Wcat", [D, 8 * H], F16, kind="ExternalInput").ap()
    bcat = nc.dram_tensor("bcat", [1, 8 * H], F16, kind="ExternalInput").ap()
    Wa = nc.dram_tensor("Wa", [2 * H, O], F16, kind="ExternalInput").ap()
    ba = nc.dram_tensor("ba", [O], F32, kind="ExternalInput").ap()
    ctxv = nc.dram_tensor("ctxv", [O], F32, kind="ExternalInput").ap()
    hT = [
        nc.dram_tensor(f"hT{d}", [H, s_len * bl], F16, kind="Internal").ap()
        for d in range(2)
    ]
    w_d = nc.dram_tensor("w_d", [bl, s_len], F32, kind="Internal").ap()
    rec_d = nc.dram_tensor("rec_d", [bl], F32, kind="Internal").ap()
    out = nc.dram_tensor("out", [bl, 2 * H], F32, kind="ExternalOutput").ap()

    with ExitStack() as ctx:
        tc = ctx.enter_context(tile.TileContext(nc))
        singles = ctx.enter_context(tc.tile_pool(name="singles", bufs=1))
        slabs = ctx.enter_context(tc.tile_pool(name="slabs", bufs=4))
        gpool = ctx.enter_context(tc.tile_pool(name="gates", bufs=8))
        tpool = ctx.enter_context(tc.tile_pool(name="tmp", bufs=12))
        psum = ctx.enter_context(tc.tile_pool(name="psum", bufs=8, space="PSUM"))

        # ---- preload weights / constants ----
        U_sb = singles.tile([H, 8 * H], F16)
        nc.sync.dma_start(out=U_sb, in_=Ucat)
        W_sb = singles.tile([D, 8 * H], F16)
        nc.sync.dma_start(out=W_sb, in_=Wcat)
        b_sb = singles.tile([1, 8 * H], F16)
        nc.sync.dma_start(out=b_sb, in_=bcat)
        ones_sb = singles.tile([1, CW], F16)
        nc.vector.memset(ones_sb, 1.0)

        # h rings (one per direction), c state ping-pong
        ring = [singles.tile([H, t_body * bl], F16, name=f"ring{d}") for d in range(2)]
        nc.vector.memset(ring[0], 0.0)
        nc.vector.memset(ring[1], 0.0)
        CT = [[singles.tile([H, bl], F32, name=f"ct{d}_{k}") for k in range(2)] for d in range(2)]
        for d in range(2):
            nc.vector.memset(CT[d][0], 0.0)
            nc.vector.memset(CT[d][1], 0.0)

        x2r = x2[:, ::-1, :]  # time-reversed view for the backward direction
        dyn = nc.sync if dyn_dma_engine == "sync" else nc.gpsimd

        # ---- the scan ----
        def scan_body(t0):
            for half in range(n_bank_pairs):
                bank = [None, None]
                for d in range(2):
                    xsrc = x2 if d == 0 else x2r
                    slab = slabs.tile([D, t_bank, bl], F16, tag="slab")
                    dyn.dma_start(
                        out=slab, in_=xsrc[:, ds(t0 + half * t_bank, t_bank), :]
                    )
                    bk = psum.tile([H, 4 * CW], F32, tag="bank")
                    bank[d] = bk
                    slab2 = slab.rearrange("p s b -> p (s b)")
                    for g in range(4):
                        c = d * 4 + g
                        # bias (rank-1) then x@W chunk; first clears the bank
                        nc.tensor.matmul(
                            bk[:, g * CW : (g + 1) * CW],
                            b_sb[:, c * H : (c + 1) * H],
                            ones_sb,
                            start=(g == 0),
                            stop=False,
                            skip_group_check=True,
                        )
                        nc.tensor.matmul(
                            bk[:, g * CW : (g + 1) * CW],
                            W_sb[:, c * H : (c + 1) * H],
                            slab2,
                            start=False,
                            stop=False,
                            skip_group_check=True,
                        )
                for s in range(t_bank):
                    j = half * t_bank + s  # step index within body
                    jp = (j - 1) % t_body
                    for d in range(2):
                        bk = bank[d]
                        rhs = ring[d][:, jp * bl : (jp + 1) * bl]
                        for g in range(4):
                            c = d * 4 + g
                            nc.tensor.matmul(
                                bk[:, g * CW + s * bl : g * CW + (s + 1) * bl],
                                U_sb[:, c * H : (c + 1) * H],
                                rhs,
                                start=False,
                                stop=(s == t_bank - 1 and g == 3),
                                skip_group_check=True,
                            )
                        gates = gpool.tile([H, 4 * bl], F32, tag="gates")
                        bk4 = bk.rearrange("p (g s b) -> p g s b", g=4, s=t_bank)
                        nc.scalar.activation(
                            out=gates.rearrange("p (g b) -> p g b", g=4),
                            in_=bk4[:, :, s, :],
                            func=AF.Sigmoid,
                        )
                        i_g = gates[:, 0:bl]
                        f_g = gates[:, bl : 2 * bl]
                        o_g = gates[:, 2 * bl : 3 * bl]
                        g_g = gates[:, 3 * bl : 4 * bl]
                        Cold = CT[d][1 - (j % 2)]
                        Cnew = CT[d][j % 2]
                        m = tpool.tile([H, bl], F32, tag="m")
                        nc.vector.scalar_tensor_tensor(
                            out=m, in0=g_g, scalar=-0.5, in1=i_g,
                            op0=ALU.add, op1=ALU.mult,
                        )
                        u = tpool.tile([H, bl], F32, tag="u")
                        nc.vector.tensor_mul(u, f_g, Cold)
                        nc.vector.tensor_add(Cnew, u, m)
                        th = tpool.tile([H, bl], F32, tag="th")
                        nc.scalar.activation(out=th, in_=Cnew, func=AF.Tanh, scale=2.0)
                        nc.vector.tensor_mul(
                            ring[d][:, j * bl : (j + 1) * bl], o_g, th
                        )
            # flush rings to DRAM h history
            hTf_v = hT[0].rearrange("p (t b) -> p t b", b=bl)
            dyn.dma_start(out=hTf_v[:, ds(t0, t_body), :], in_=ring[0])
            hTb_v = hT[1].rearrange("p (t b) -> p t b", b=bl)[:, ::-1, :]
            dyn.dma_start(out=hTb_v[:, ds(t0, t_body), :], in_=ring[1])

        if unroll:
            for t0 in range(0, s_len, t_body):
                scan_body(t0)
        else:
            with tc.For_i(0, s_len, t_body) as t0:
                scan_body(t0)

        # ---- attention phase ----
        N = s_len * bl
        hT_sb = [singles.tile([H, N], F16, name=f"hTsb{d}") for d in range(2)]
        nc.sync.dma_start(out=hT_sb[0], in_=hT[0])
        nc.sync.dma_start(out=hT_sb[1], in_=hT[1])
        Wa_sb = [singles.tile([H, O], F16, name=f"Wasb{k}") for k in range(2)]
        for kc in range(2):
            nc.sync.dma_start(out=Wa_sb[kc], in_=Wa[kc * H : (kc + 1) * H, :])
        ba_sb = singles.tile([H, 2], F32)
        nc.sync.dma_start(out=ba_sb, in_=_ap(ba, 0, [[1, H], [H, 2]]))
        ctx_sb = singles.tile([H, 2], F32)
        nc.sync.dma_start(out=ctx_sb, in_=_ap(ctxv, 0, [[1, H], [H, 2]]))
        scores = singles.tile([1, N], F32)

        NG = N // 512
        for gidx in range(NG):
            scp = psum.tile([1, 512], F32, tag="bank")
            for oc in range(2):
                pp = psum.tile([H, 512], F32, tag="bank")
                for kc in range(2):
                    nc.tensor.matmul(
                        pp,
                        Wa_sb[kc][:, oc * H : (oc + 1) * H],
                        hT_sb[kc][:, gidx * 512 : (gidx + 1) * 512],
                        start=(kc == 0),
                        stop=(kc == 1),
                        skip_group_check=True,
                    )
                proj = gpool.tile([H, 512], F32, tag="proj")
                nc.scalar.activation(
                    out=proj, in_=pp, func=AF.Tanh, bias=ba_sb[:, oc : oc + 1]
                )
                nc.tensor.matmul(
                    scp,
                    ctx_sb[:, oc : oc + 1],
                    proj,
                    start=(oc == 0),
                    stop=(oc == 1),
                    skip_group_check=True,
                )
            nc.vector.tensor_copy(scores[:, gidx * 512 : (gidx + 1) * 512], scp)

        # scores [1,(t,b)] -> [bl, s_len] ; softmax over t per batch row
        sct = singles.tile([bl, s_len], F32)
        scores_v = scores.rearrange("p (t b) -> p t b", b=bl)
        for b in range(bl):
            nc.sync.dma_start(
                out=sct[b : b + 1, :], in_=scores_v[:, :, b : b + 1]
            )
        mx = tpool.tile([bl, 1], F32, tag="sm")
        nc.vector.tensor_reduce(out=mx, in_=sct, axis=mybir.AxisListType.X, op=ALU.max)
        nmx = tpool.tile([bl, 1], F32, tag="sm")
        nc.vector.tensor_scalar_mul(nmx, mx, -1.0)
        w = singles.tile([bl, s_len], F32)
        nc.scalar.activation(out=w, in_=sct, func=AF.Exp, bias=nmx)
        sm = tpool.tile([bl, 1], F32, tag="sm")
        nc.vector.tensor_reduce(out=sm, in_=w, axis=mybir.AxisListType.X, op=ALU.add)
        rec = tpool.tile([bl, 1], F32, tag="sm")
        nc.vector.reciprocal(rec, sm)
        nc.sync.dma_start(out=w_d, in_=w)
        nc.sync.dma_start(out=rec_d, in_=rec)
        recb = tpool.tile([H, bl], F32, tag="recb")
        nc.sync.dma_start(out=recb, in_=_ap(rec_d, 0, [[0, H], [1, bl]]))

        wexp = singles.tile([H, s_len], F32)
        scratch = singles.tile([H, s_len], F32)
        outT = [singles.tile([H, bl], F32, name=f"outT{k}") for k in range(2)]
        for b in range(bl):
            nc.sync.dma_start(
                out=wexp, in_=_ap(w_d, b * s_len, [[0, H], [1, s_len]])
            )
            for kc in range(2):
                src = hT_sb[kc].rearrange("p (t b) -> p t b", b=bl)[:, :, b]
                nc.vector.scalar_tensor_tensor(
                    out=scratch,
                    in0=src,
                    scalar=1.0,
                    in1=wexp,
                    op0=ALU.mult,
                    op1=ALU.mult,
                    accum_out=outT[kc][:, b : b + 1],
                )
        for kc in range(2):
            outn = tpool.tile([H, bl], F32, tag="outn")
            nc.vector.tensor_mul(outn, outT[kc], recb)
            nc.sync.dma_start(
                out=_ap(out, kc * H, [[1, H], [2 * H, bl]]), in_=outn
            )

    nc.compile()
    return nc


def get_program(key=(S, BL)):
    if key not in _PROGRAM_CACHE:
        _PROGRAM_CACHE[key] = build_program(s_len=key[0], bl=key[1], unroll=True)
    return _PROGRAM_CACHE[key]


def prep_shared(Wf, Uf, bf, Wb, Ub, bb, attn_W, attn_b, ctx):
    """Host-side weight prep: gate reorder (i,f,o,g), g-gate x2, fp16 casts."""
    def cat8(Af, Ab):
        # reference gate order along 4H: i,f,g,o -> ours per dir: i,f,o,g (g x2)
        blocks = []
        for Asrc in (Af, Ab):
            i_b = Asrc[..., 0 * H : 1 * H]
            f_b = Asrc[..., 1 * H : 2 * H]
            g_b = Asrc[..., 2 * H : 3 * H]
            o_b = Asrc[..., 3 * H : 4 * H]
            blocks += [i_b, f_b, o_b, 2.0 * g_b]
        return np.concatenate(blocks, axis=-1)

    return {
        "Ucat": cat8(Uf, Ub).astype(np.float16),
        "Wcat": cat8(Wf, Wb).astype(np.float16),
        "bcat": cat8(bf[None, :], bb[None, :]).astype(np.float16),
        "Wa": np.asarray(attn_W, np.float32).astype(np.float16),
        "ba": np.asarray(attn_b, np.float32),
        "ctxv": np.asarray(ctx, np.float32),
    }


def make_in_maps(x, Wf, Uf, bf, Wb, Ub, bb, attn_W, attn_b, ctx):
    x = np.asarray(x, np.float32)
    shared = prep_shared(
        np.asarray(Wf, np.float32), np.asarray(Uf, np.float32),
        np.asarray(bf, np.float32), np.asarray(Wb, np.float32),
        np.asarray(Ub, np.float32), np.asarray(bb, np.float32),
        np.asarray(attn_W, np.float32), np.asarray(attn_b, np.float32),
        np.asarray(ctx, np.float32),
    )
    in_maps = []
    for k in range(NCORES):
        xs = x[k * BL : (k + 1) * BL]  # [BL, S, D]
        x2 = np.ascontiguousarray(xs.transpose(2, 1, 0)).astype(np.float16)
        m = dict(shared)
        m["x2"] = x2
        in_maps.append(m)
    return in_maps


def run_cores(in_maps, trace=False, **kwargs):
    from concourse.bass_utils import run_bass_kernel_spmd

    nc = get_program()
    return run_bass_kernel_spmd(
        nc, in_maps, core_ids=list(range(NCORES)), trace=trace, **kwargs
    )


def kernel(**inputs):
    res = run_cores(make_in_maps(**inputs))
    outs = [res.results[k]["out"] for k in range(NCORES)]
    return np.concatenate(outs, axis=0).astype(np.float32)


# revision 16
# speedup vs baseline: 975.2790x; 975.2790x over previous
"""BiLSTM + additive-attention pooling kernel for Trainium2 (8 NeuronCores).

Strategy (data-parallel over batch, per spec sharding hint):
  - B=64 batch rows sharded 8-ways -> BL=8 rows per core; LSTM/attention
    weights replicated.  Each core runs BOTH directions of the LSTM as two
    independent instruction chains (latency hiding) over S=2048 steps.
  - Transposed layout everywhere: hidden dim on SBUF partitions, batch on the
    free dim.  Recurrent matmuls: z_G^T = U_G^T @ h^T with U gate-chunks as
    the stationary operand (fp16 -> fast weight load), h^T (fp16) streaming.
  - The input projection x@W and the bias are folded into the same PSUM banks
    the recurrent matmuls accumulate into:  per 16-step bank we first do a
    rank-1 bias matmul (start=True) + W-chunk matmuls over an x slab, then the
    per-step U-chunk matmuls land on top with start=False.  No xW round-trip
    through DRAM, no extra vector-engine adds.
  - Gates use the sigmoid-only trick: tanh(z) = 2*sigmoid(2z)-1, with the
    factor 2 folded into the g-gate columns of U/W/b on the host.  One
    sigmoid ACT per (dir, step) covers all 4 gates; the cell state is tracked
    halved (C = c/2) so the correction is a single fused scalar_tensor_tensor:
        m   = (sig_g - 0.5) * i          # = i*g/2
        u   = f * C_prev                 # C' = f*C_prev + m
        th  = tanh(2*C')                 # = tanh(c)  (ACT, scale=2)
        h   = o * th                     # fp16, feeds next matmul + attention
  - h^T for all t is spilled to DRAM (fp16) in 64-step rings, then the
    attention phase (big matmuls, softmax, weighted sum) runs on-chip.

kernel(**inputs) takes the FULL fp32 inputs and returns the FULL [64, 256]
fp32 output; it shards/preps on the host, runs the same program SPMD on cores
0-7 via run_bass_kernel_spmd, and concatenates the per-core outputs.
"""

import sys

for p in ("/opt/trn_rl_repo", "/opt/pypackages"):
    if p not in sys.path:
        sys.path.insert(0, p)

from contextlib import ExitStack

import numpy as np

import concourse.bass as bass
import concourse.tile as tile
from concourse import bacc, mybir
from concourse.bass import ds

F32 = mybir.dt.float32
F16 = mybir.dt.float16
AF = mybir.ActivationFunctionType
ALU = mybir.AluOpType

B, S, D, H, O = 64, 2048, 128, 128, 256
NCORES = 8
BL = B // NCORES  # batch rows per core

_PROGRAM_CACHE = {}


def _ap(t, offset, ap):
    """Manual access-pattern helper."""
    return bass.AP(tensor=t.tensor, offset=t.offset + offset, ap=ap)


def build_program(s_len=S, bl=BL, t_bank=16, n_bank_pairs=2, unroll=False,
                  dyn_dma_engine="sync"):
    """Build the per-core Bass program.

    s_len steps, bl batch rows.  Each PSUM bank holds t_bank steps of one
    direction; one For_i iteration covers n_bank_pairs banks per direction
    (t_body = t_bank * n_bank_pairs steps).  unroll=True replaces the
    dynamic For_i loop with a fully unrolled python loop (static offsets).
    """
    t_body = t_bank * n_bank_pairs
    assert s_len % t_body == 0
    iters = s_len // t_body
    CW = t_bank * bl  # psum cols per gate chunk (16*8 = 128)

    nc = bacc.Bacc("TRN2", target_bir_lowering=False, debug=False)

    x2 = nc.dram_tensor("x2", [D, s_len, bl], F16, kind="ExternalInput").ap()
    Ucat = nc.dram_tensor("Ucat", [H, 8 * H], F16, kind="ExternalInput").ap()
    Wcat = nc.dram_tensor("Wcat", [D, 8 * H], F16, kind="ExternalInput").ap()
    bcat = nc.dram_tensor("bcat", [1, 8 * H], F16, kind="ExternalInput").ap()
    Wa = nc.dram_tensor("Wa", [2 * H, O], F16, kind="ExternalInput").ap()
    ba = nc.dram_tensor("ba", [O], F32, kind="ExternalInput").ap()
    ctxv = nc.dram_tensor("ctxv", [O], F32, kind="ExternalInput").ap()
    hT = [
        nc.dram_tensor(f"hT{d}", [H, s_len * bl], F16, kind="Internal").ap()
        for d in range(2)
    ]
    w_d = nc.dram_tensor("w_d", [bl, s_len], F32, kind="Internal").ap()
    rec_d = nc.dram_tensor("rec_d", [bl], F32, kind="Internal").ap()
    out = nc.dram_tensor("out", [bl, 2 * H], F32, kind="ExternalOutput").ap()

    with ExitStack() as ctx:
        tc = ctx.enter_context(tile.TileContext(nc))
        singles = ctx.enter_context(tc.tile_pool(name="singles", bufs=1))
        slabs = ctx.enter_context(tc.tile_pool(name="slabs", bufs=4))
        gpool = ctx.enter_context(tc.tile_pool(name="gates", bufs=8))
        tpool = ctx.enter_context(tc.tile_pool(name="tmp", bufs=12))
        psum = ctx.enter_context(tc.tile_pool(name="psum", bufs=8, space="PSUM"))

        # ---- preload weights / constants ----
        U_sb = singles.tile([H, 8 * H], F16)
        nc.sync.dma_start(out=U_sb, in_=Ucat)
        W_sb = singles.tile([D, 8 * H], F16)
        nc.sync.dma_start(out=W_sb, in_=Wcat)
        b_sb = singles.tile([1, 8 * H], F16)
        nc.sync.dma_start(out=b_sb, in_=bcat)
        ones_sb = singles.tile([1, CW], F16)
        nc.vector.memset(ones_sb, 1.0)

        # h rings (one per direction), c state ping-pong
        ring = [singles.tile([H, t_body * bl], F16, name=f"ring{d}") for d in range(2)]
        nc.vector.memset(ring[0], 0.0)
        nc.vector.memset(ring[1], 0.0)
        CT = [[singles.tile([H, bl], F32, name=f"ct{d}_{k}") for k in range(2)] for d in range(2)]
        for d in range(2):
            nc.vector.memset(CT[d][0], 0.0)
            nc.vector.memset(CT[d][1], 0.0)

        x2r = x2[:, ::-1, :]  # time-reversed view for the backward direction
        dyn = nc.sync if dyn_dma_engine == "sync" else nc.gpsimd

        # ---- the scan ----
        def scan_body(t0):
            for half in range(n_bank_pairs):
                bank = [None, None]
                for d in range(2):
                    xsrc = x2 if d == 0 else x2r
                    slab = slabs.tile([D, t_bank, bl], F16, tag="slab")
                    dyn.dma_start(
                        out=slab, in_=xsrc[:, ds(t0 + half * t_bank, t_bank), :]
                    )
                    bk = psum.tile([H, 4 * CW], F32, tag="bank")
                    bank[d] = bk
                    slab2 = slab.rearrange("p s b -> p (s b)")
                    for g in range(4):
                        c = d * 4 + g
                        # bias (rank-1) then x@W chunk; first clears the bank
                        nc.tensor.matmul(
                            bk[:, g * CW : (g + 1) * CW],
                            b_sb[:, c * H : (c + 1) * H],
                            ones_sb,
                            start=(g == 0),
                            stop=False,
                            skip_group_check=True,
                        )
                        nc.tensor.matmul(
                            bk[:, g * CW : (g + 1) * CW],
                            W_sb[:, c * H : (c + 1) * H],
                            slab2,
                            start=False,
                            stop=False,
                            skip_group_check=True,
                        )
                for s in range(t_bank):
                    j = half * t_bank + s  # step index within body
                    jp = (j - 1) % t_body
                    # interleave the two directions stage-by-stage so their
                    # dependency chains pipeline through the FIFO engines
                    for d in range(2):
                        rhs = ring[d][:, jp * bl : (jp + 1) * bl]
                        for g in range(4):
                            c = d * 4 + g
                            nc.tensor.matmul(
                                bank[d][:, g * CW + s * bl : g * CW + (s + 1) * bl],
                                U_sb[:, c * H : (c + 1) * H],
                                rhs,
                                start=False,
                                stop=(s == t_bank - 1 and g == 3),
                                skip_group_check=True,
                            )
                    gates = []
                    for d in range(2):
                        gt = gpool.tile([H, 4 * bl], F32, tag="gates", name=f"g{d}")
                        bk4 = bank[d].rearrange(
                            "p (g s b) -> p g s b", g=4, s=t_bank
                        )
                        nc.scalar.activation(
                            out=gt.rearrange("p (g b) -> p g b", g=4),
                            in_=bk4[:, :, s, :],
                            func=AF.Sigmoid,
                        )
                        gates.append(gt)
                    m, u = [], []
                    for d in range(2):
                        mt = tpool.tile([H, bl], F32, tag="m", name=f"m{d}")
                        nc.vector.scalar_tensor_tensor(
                            out=mt, in0=gates[d][:, 3 * bl : 4 * bl], scalar=-0.5,
                            in1=gates[d][:, 0:bl], op0=ALU.add, op1=ALU.mult,
                        )
                        m.append(mt)
                    for d in range(2):
                        ut = tpool.tile([H, bl], F32, tag="u", name=f"u{d}")
                        nc.vector.tensor_mul(
                            ut, gates[d][:, bl : 2 * bl], CT[d][1 - (j % 2)]
                        )
                        u.append(ut)
                    for d in range(2):
                        nc.vector.tensor_add(CT[d][j % 2], u[d], m[d])
                    th = []
                    for d in range(2):
                        tt = tpool.tile([H, bl], F32, tag="th", name=f"th{d}")
                        nc.scalar.activation(
                            out=tt, in_=CT[d][j % 2], func=AF.Tanh, scale=2.0
                        )
                        th.append(tt)
                    for d in range(2):
                        nc.vector.tensor_mul(
                            ring[d][:, j * bl : (j + 1) * bl],
                            gates[d][:, 2 * bl : 3 * bl], th[d]
                        )
            # flush rings to DRAM h history
            hTf_v = hT[0].rearrange("p (t b) -> p t b", b=bl)
            dyn.dma_start(out=hTf_v[:, ds(t0, t_body), :], in_=ring[0])
            hTb_v = hT[1].rearrange("p (t b) -> p t b", b=bl)[:, ::-1, :]
            dyn.dma_start(out=hTb_v[:, ds(t0, t_body), :], in_=ring[1])

        if unroll:
            for t0 in range(0, s_len, t_body):
                scan_body(t0)
        else:
            with tc.For_i(0, s_len, t_body) as t0:
                scan_body(t0)

        # ---- attention phase ----
        N = s_len * bl
        hT_sb = [singles.tile([H, N], F16, name=f"hTsb{d}") for d in range(2)]
        nc.sync.dma_start(out=hT_sb[0], in_=hT[0])
        nc.sync.dma_start(out=hT_sb[1], in_=hT[1])
        Wa_sb = [singles.tile([H, O], F16, name=f"Wasb{k}") for k in range(2)]
        for kc in range(2):
            nc.sync.dma_start(out=Wa_sb[kc], in_=Wa[kc * H : (kc + 1) * H, :])
        ba_sb = singles.tile([H, 2], F32)
        nc.sync.dma_start(out=ba_sb, in_=_ap(ba, 0, [[1, H], [H, 2]]))
        ctx_sb = singles.tile([H, 2], F32)
        nc.sync.dma_start(out=ctx_sb, in_=_ap(ctxv, 0, [[1, H], [H, 2]]))
        scores = singles.tile([1, N], F32)

        NG = N // 512
        for gidx in range(NG):
            scp = psum.tile([1, 512], F32, tag="bank")
            for oc in range(2):
                pp = psum.tile([H, 512], F32, tag="bank")
                for kc in range(2):
                    nc.tensor.matmul(
                        pp,
                        Wa_sb[kc][:, oc * H : (oc + 1) * H],
                        hT_sb[kc][:, gidx * 512 : (gidx + 1) * 512],
                        start=(kc == 0),
                        stop=(kc == 1),
                        skip_group_check=True,
                    )
                proj = gpool.tile([H, 512], F32, tag="proj")
                nc.scalar.activation(
                    out=proj, in_=pp, func=AF.Tanh, bias=ba_sb[:, oc : oc + 1]
                )
                nc.tensor.matmul(
                    scp,
                    ctx_sb[:, oc : oc + 1],
                    proj,
                    start=(oc == 0),
                    stop=(oc == 1),
                    skip_group_check=True,
                )
            nc.vector.tensor_copy(scores[:, gidx * 512 : (gidx + 1) * 512], scp)

        # scores [1,(t,b)] -> [bl, s_len] ; softmax over t per batch row
        sct = singles.tile([bl, s_len], F32)
        scores_v = scores.rearrange("p (t b) -> p t b", b=bl)
        for b in range(bl):
            nc.sync.dma_start(
                out=sct[b : b + 1, :], in_=scores_v[:, :, b : b + 1]
            )
        mx = tpool.tile([bl, 1], F32, tag="sm")
        nc.vector.tensor_reduce(out=mx, in_=sct, axis=mybir.AxisListType.X, op=ALU.max)
        nmx = tpool.tile([bl, 1], F32, tag="sm")
        nc.vector.tensor_scalar_mul(nmx, mx, -1.0)
        w = singles.tile([bl, s_len], F32)
        nc.scalar.activation(out=w, in_=sct, func=AF.Exp, bias=nmx)
        sm = tpool.tile([bl, 1], F32, tag="sm")
        nc.vector.tensor_reduce(out=sm, in_=w, axis=mybir.AxisListType.X, op=ALU.add)
        rec = tpool.tile([bl, 1], F32, tag="sm")
        nc.vector.reciprocal(rec, sm)
        nc.sync.dma_start(out=w_d, in_=w)
        nc.sync.dma_start(out=rec_d, in_=rec)
        recb = tpool.tile([H, bl], F32, tag="recb")
        nc.sync.dma_start(out=recb, in_=_ap(rec_d, 0, [[0, H], [1, bl]]))

        wexp = singles.tile([H, s_len], F32)
        scratch = singles.tile([H, s_len], F32)
        outT = [singles.tile([H, bl], F32, name=f"outT{k}") for k in range(2)]
        for b in range(bl):
            nc.sync.dma_start(
                out=wexp, in_=_ap(w_d, b * s_len, [[0, H], [1, s_len]])
            )
            for kc in range(2):
                src = hT_sb[kc].rearrange("p (t b) -> p t b", b=bl)[:, :, b]
                nc.vector.scalar_tensor_tensor(
                    out=scratch,
                    in0=src,
                    scalar=1.0,
                    in1=wexp,
                    op0=ALU.mult,
                    op1=ALU.mult,
                    accum_out=outT[kc][:, b : b + 1],
                )
        for kc in range(2):
            outn = tpool.tile([H, bl], F32, tag="outn")
            nc.vector.tensor_mul(outn, outT[kc], recb)
            nc.sync.dma_start(
                out=_ap(out, kc * H, [[1, H], [2 * H, bl]]), in_=outn
            )

    nc.compile()
    return nc


def get_program(key=(S, BL)):
    if key not in _PROGRAM_CACHE:
        _PROGRAM_CACHE[key] = build_program(s_len=key[0], bl=key[1], unroll=True)
    return _PROGRAM_CACHE[key]


def prep_shared(Wf, Uf, bf, Wb, Ub, bb, attn_W, attn_b, ctx):
    """Host-side weight prep: gate reorder (i,f,o,g), g-gate x2, fp16 casts."""
    def cat8(Af, Ab):
        # reference gate order along 4H: i,f,g,o -> ours per dir: i,f,o,g (g x2)
        blocks = []
        for Asrc in (Af, Ab):
            i_b = Asrc[..., 0 * H : 1 * H]
            f_b = Asrc[..., 1 * H : 2 * H]
            g_b = Asrc[..., 2 * H : 3 * H]
            o_b = Asrc[..., 3 * H : 4 * H]
            blocks += [i_b, f_b, o_b, 2.0 * g_b]
        return np.concatenate(blocks, axis=-1)

    return {
        "Ucat": cat8(Uf, Ub).astype(np.float16),
        "Wcat": cat8(Wf, Wb).astype(np.float16),
        "bcat": cat8(bf[None, :], bb[None, :]).astype(np.float16),
        "Wa": np.asarray(attn_W, np.float32).astype(np.float16),
        "ba": np.asarray(attn_b, np.float32),
        "ctxv": np.asarray(ctx, np.float32),
    }


def make_in_maps(x, Wf, Uf, bf, Wb, Ub, bb, attn_W, attn_b, ctx):
    x = np.asarray(x, np.float32)
    shared = prep_shared(
        np.asarray(Wf, np.float32), np.asarray(Uf, np.float32),
        np.asarray(bf, np.float32), np.asarray(Wb, np.float32),
        np.asarray(Ub, np.float32), np.asarray(bb, np.float32),
        np.asarray(attn_W, np.float32), np.asarray(attn_b, np.float32),
        np.asarray(ctx, np.float32),
    )
    in_maps = []
    for k in range(NCORES):
        xs = x[k * BL : (k + 1) * BL]  # [BL, S, D]
        x2 = np.ascontiguousarray(xs.transpose(2, 1, 0)).astype(np.float16)
        m = dict(shared)
        m["x2"] = x2
        in_maps.append(m)
    return in_maps


def run_cores(in_maps, trace=False, **kwargs):
    from concourse.bass_utils import run_bass_kernel_spmd

    nc = get_program()
    return run_bass_kernel_spmd(
        nc, in_maps, core_ids=list(range(NCORES)), trace=trace, **kwargs
    )


_RUNNER = {}


def get_runner():
    """Cached jitted SPMD executable (compiles the NEFF once per process)."""
    if "fn" in _RUNNER:
        return _RUNNER["fn"]
    import jax
    import jax.numpy as jnp
    from jax.experimental.shard_map import shard_map
    from jax.sharding import Mesh, PartitionSpec
    from concourse import bass2jax, mybir as _mb

    nc = get_program()
    bass2jax.install_neuronx_cc_hook()

    in_names, out_names, out_avals, zero_outs = [], [], [], []
    partition_name = nc.partition_id_tensor.name if nc.partition_id_tensor else None
    for alloc in nc.m.functions[0].allocations:
        if not isinstance(alloc, _mb.MemoryLocationSet):
            continue
        name = alloc.memorylocations[0].name
        if alloc.kind == "ExternalInput":
            if name != partition_name:
                in_names.append(name)
        elif alloc.kind == "ExternalOutput":
            shape = tuple(alloc.tensor_shape)
            dtype = _mb.dt.np(alloc.dtype)
            out_names.append(name)
            out_avals.append(jax.core.ShapedArray(shape, dtype))
            zero_outs.append(np.zeros(shape, dtype))
    n_params, n_outs = len(in_names), len(out_avals)
    all_names = in_names + out_names
    if partition_name is not None:
        all_names = all_names + [partition_name]
    donate = tuple(range(n_params, n_params + n_outs))

    def _body(*args):
        operands = list(args)
        if partition_name is not None:
            operands.append(bass2jax.partition_id_tensor())
        return tuple(
            bass2jax._bass_exec_p.bind(
                *operands,
                out_avals=tuple(out_avals),
                in_names=tuple(all_names),
                out_names=tuple(out_names),
                lowering_input_output_aliases=(),
                sim_require_finite=True,
                sim_require_nnan=True,
                nc=nc,
            )
        )

    devices = jax.devices()[:NCORES]
    mesh = Mesh(np.asarray(devices), ("core",))
    specs = (PartitionSpec("core"),) * (n_params + n_outs)
    sharded = jax.jit(
        shard_map(_body, mesh=mesh, in_specs=specs,
                  out_specs=(PartitionSpec("core"),) * n_outs, check_rep=False),
        donate_argnums=donate, keep_unused=True,
    )

    def run(in_maps):
        concat_in = [
            np.concatenate([np.asarray(m[name]) for m in in_maps], axis=0)
            for name in in_names
        ]
        concat_zero = [
            np.zeros((NCORES * z.shape[0], *z.shape[1:]), z.dtype)
            for z in zero_outs
        ]
        outs = sharded(*concat_in, *concat_zero)
        o = np.asarray(outs[out_names.index("out")])
        return o.reshape(NCORES, -1, o.shape[-1])

    _RUNNER["fn"] = run
    return run


def kernel(**inputs):
    run = get_runner()
    out = run(make_in_maps(**inputs))
    return np.concatenate(list(out), axis=0).astype(np.float32)
